# revision 1
# baseline (speedup 1.0000x reference)
"""Trainium2 Bass kernel for nn_ChannelMixing (RWKV-style channel mixing).

Math: the reference's FFT decay-conv is the first-order IIR
    h[t] = mix*h[t-1] + x[t],  h[-1] = last_x/(1-mix)
and x_mix = (1-mix)*h, so with weights pre-scaled by (1-mix):
    k = h_k @ (Wk*(1-mix_k)).T,  r = h_r @ (Wr*(1-mix_r)).T
    out = sigmoid(r) * (relu(k)^2 @ Wv.T)

Sharding: time dimension L=4096 split over 8 cores (512 rows each) with a
64-step halo to warm up the scan state (decay <= sigmoid(1) ~ 0.731, so
carry across 64 steps < 3e-9 — below fp32 noise). Core 0 gets the exact
initial state via a per-core init column; no collectives.

Layout: everything [channel(P), time(F)]. The scan runs on the vector
engine (tensor_tensor_scan), the three 2048x2048 matmuls on the PE in
fp32r, activations on ACT, gating on DVE.
"""
import numpy as np
from contextlib import ExitStack

import concourse.bass as bass
from concourse import bacc
import concourse.tile as tile
import concourse.mybir as mybir
from concourse.bass_utils import run_bass_kernel_spmd

LEN, DIM = 4096, 2048
NCORES = 8
P = 128
HALO = 64

f32 = mybir.dt.float32
f32r = mybir.dt.float32r
Alu = mybir.AluOpType
Act = mybir.ActivationFunctionType

_cache = {}


def _build(dim, tloc, halo):
    """Build + compile the per-core SPMD program."""
    nt = dim // P          # channel tiles
    ts = tloc + halo       # time slab incl. halo
    ng = max(1, (dim // P) // 4)   # output m-groups of 4 m-tiles
    NF = 512 if tloc >= 512 else tloc   # matmul moving size (time)
    assert tloc % NF == 0
    nf = tloc // NF        # time blocks per matmul (1 at full size)

    nc = bacc.Bacc(trn_type="TRN2", debug=False)

    xs_d = nc.dram_tensor("xs", [dim, ts], f32, kind="ExternalInput").ap()
    dec_d = nc.dram_tensor("dec", [P, 2 * nt], f32, kind="ExternalInput").ap()  # SBUF image
    ini_d = nc.dram_tensor("ini", [P, 2 * nt], f32, kind="ExternalInput").ap()
    wk_d = nc.dram_tensor("wk", [dim, dim], f32r, kind="ExternalInput").ap()  # [d, i] pre-scaled
    wr_d = nc.dram_tensor("wr", [dim, dim], f32r, kind="ExternalInput").ap()
    wv_d = nc.dram_tensor("wv", [dim, dim], f32r, kind="ExternalInput").ap()  # [i, o]
    out_d = nc.dram_tensor("out", [dim, tloc], f32, kind="ExternalOutput").ap()

    with tile.TileContext(nc) as tc, ExitStack() as ctx:
        const = ctx.enter_context(tc.tile_pool(name="const", bufs=1))
        xs_pool = ctx.enter_context(tc.tile_pool(name="xs", bufs=6))
        h_pool = ctx.enter_context(tc.tile_pool(name="h", bufs=1))
        w_pool = ctx.enter_context(tc.tile_pool(name="w", bufs=12))
        ev_pool = ctx.enter_context(tc.tile_pool(name="ev", bufs=1))
        sc_pool = ctx.enter_context(tc.tile_pool(name="sc", bufs=3))
        o_pool = ctx.enter_context(tc.tile_pool(name="o", bufs=3))
        ps_pool = ctx.enter_context(tc.tile_pool(name="ps", bufs=2, space="PSUM"))

        # per-channel constants: [P, nt] tiles (col ct = chan tile ct)
        dec_t = const.tile([P, 2 * nt], f32)
        nc.scalar.dma_start(dec_t[:], dec_d)
        ini_t = const.tile([P, 2 * nt], f32)
        nc.scalar.dma_start(ini_t[:], ini_d)

        # ---- stage A: decay scans -> h_k, h_r in [chan, time] ----
        h = {"k": [None] * nt, "r": [None] * nt}
        for pi, p in enumerate(("k", "r")):
            for ct in range(nt):
                xs = xs_pool.tile([P, ts], f32, tag="xs", name=f"xs{p}{ct}")
                nc.scalar.dma_start(xs[:], xs_d[ct * P:(ct + 1) * P, :])
                dcol = dec_t[:, 2 * ct + pi: 2 * ct + pi + 1]
                # single scan over halo+body; core0's initial state is
                # h0*mix^-halo (host-prepped) so it decays to exactly h0
                # across the zero halo columns.
                hs = h_pool.tile([P, ts], f32r, tag=f"h{p}{ct}", name=f"hs{p}{ct}")
                nc.vector.tensor_tensor_scan(
                    hs[:], dcol.broadcast_to([P, ts]), xs[:],
                    ini_t[:, 2 * ct + pi: 2 * ct + pi + 1],
                    op0=Alu.mult, op1=Alu.add)
                h[p][ct] = hs[:, halo:]

        # ---- stage B helper: out[o_tile, t] = sum_kt w[kt,o].T @ rhs[kt] ----
        def big_matmul(w_dram, rhs_tiles, evict_fn, wtag):
            for g in range(ng):
                m4 = min(4, nt - 4 * g)
                psums = [ps_pool.tile([P, NF], f32, tag=f"ps{m}",
                                      name=f"ps_{wtag}_{g}_{m}") for m in range(m4)]
                for tb in range(nf):
                    for kt in range(nt):
                        wt = w_pool.tile([P, m4 * P], f32r, tag="w",
                                         name=f"wt_{wtag}_{g}_{kt}")
                        nc.sync.dma_start(
                            wt[:], w_dram[kt * P:(kt + 1) * P,
                                          g * 4 * P: g * 4 * P + m4 * P])
                        for m in range(m4):
                            nc.tensor.matmul(
                                psums[m][:], wt[:, m * P:(m + 1) * P],
                                rhs_tiles[kt][:, tb * NF:(tb + 1) * NF],
                                start=(kt == 0), stop=(kt == nt - 1))
                    for m in range(m4):
                        evict_fn(g * 4 + m, tb, psums[m])

        # k path: evict = relu then square -> sq tiles (f32r)
        sq = [ev_pool.tile([P, tloc], f32r, tag=f"sq{i}", name=f"sq{i}") for i in range(nt)]

        def evict_k(mi, tb, psum):
            rr = sc_pool.tile([P, NF], f32, tag="rr")
            nc.scalar.activation(rr[:], psum[:], Act.Relu)
            nc.vector.tensor_mul(sq[mi][:, tb * NF:(tb + 1) * NF], rr[:], rr[:])

        # r path: evict = sigmoid -> sig tiles (f32)
        sig = [ev_pool.tile([P, tloc], f32, tag=f"sg{i}", name=f"sg{i}") for i in range(nt)]

        def evict_r(mi, tb, psum):
            nc.scalar.activation(sig[mi][:, tb * NF:(tb + 1) * NF], psum[:], Act.Sigmoid)

        # v path: evict = gate with sigmoid(r) -> DMA out
        def evict_v(mi, tb, psum):
            ot = o_pool.tile([P, NF], f32, tag="ot")
            nc.vector.tensor_mul(ot[:], psum[:], sig[mi][:, tb * NF:(tb + 1) * NF])
            nc.sync.dma_start(out_d[mi * P:(mi + 1) * P, tb * NF:(tb + 1) * NF], ot[:])

        # PE warmup during the scan phase: keeps HAM at K=8/8 so the real
        # matmul stream starts warm. Uses the first weight tile as both
        # operands; results are discarded (psum slot reused with start=True).
        wsz = min(NF, dim)
        wm = min(P, wsz)
        wt0 = w_pool.tile([P, wsz], f32r, tag="w", name="wt_warm")
        nc.sync.dma_start(wt0[:], wk_d[0:P, 0:wsz])
        ps_w = ps_pool.tile([P, wsz], f32, tag="ps0", name="ps_warm")
        for _ in range(28):
            nc.tensor.matmul(ps_w[0:wm, :], wt0[:, 0:wm],
                             wt0[:], start=True, stop=True)

        big_matmul(wk_d, h["k"], evict_k, "wk")
        big_matmul(wr_d, h["r"], evict_r, "wr")
        big_matmul(wv_d, sq, evict_v, "wv")

    nc.compile()
    return nc


def _sigmoid(v):
    return 1.0 / (1.0 + np.exp(-v.astype(np.float64)))


def _prep(x, Wk, Wr, Wv, mix_k, mix_r, lxk, lxr, ncores, halo):
    """Host-side prep: transposes, weight pre-scaling, per-core slabs."""
    dim = x.shape[1]
    tloc = x.shape[0] // ncores
    mk = _sigmoid(mix_k).astype(np.float32)
    mr = _sigmoid(mix_r).astype(np.float32)
    h0k = (lxk / (1.0 - mk)).astype(np.float32)
    h0r = (lxr / (1.0 - mr)).astype(np.float32)
    P = 128
    nt = dim // P
    dec = np.empty((P, 2 * nt), np.float32)   # SBUF image: [p, 2*ct+path]
    dec[:, 0::2] = mk.reshape(nt, P).T
    dec[:, 1::2] = mr.reshape(nt, P).T

    wk = np.ascontiguousarray((Wk * (1.0 - mk)[None, :]).T.astype(np.float32))
    wr = np.ascontiguousarray((Wr * (1.0 - mr)[None, :]).T.astype(np.float32))
    wv = np.ascontiguousarray(Wv.T.astype(np.float32))

    xT = np.ascontiguousarray(x.T.astype(np.float32))       # [dim, L]
    in_maps = []
    for c in range(ncores):
        t0 = c * tloc
        slab = np.empty((dim, halo + tloc), np.float32)
        if c == 0:
            slab[:, :halo] = 0.0
            bk = (h0k.astype(np.float64) * (1.0 / mk.astype(np.float64)) ** halo
                  ).astype(np.float32)
            br = (h0r.astype(np.float64) * (1.0 / mr.astype(np.float64)) ** halo
                  ).astype(np.float32)
            ini = np.empty((P, 2 * nt), np.float32)
            ini[:, 0::2] = bk.reshape(nt, P).T
            ini[:, 1::2] = br.reshape(nt, P).T
        else:
            slab[:, :halo] = xT[:, t0 - halo: t0]
            ini = np.zeros((P, 2 * nt), np.float32)
        slab[:, halo:] = xT[:, t0: t0 + tloc]
        in_maps.append({
            "xs": slab, "dec": dec, "ini": np.ascontiguousarray(ini),
            "wk": wk, "wr": wr, "wv": wv,
        })
    return in_maps


def kernel(x, Wk, Wr, Wv, mix_k, mix_r, last_x_mix_k, last_x_mix_r):
    x = np.asarray(x, np.float32)
    Wk = np.asarray(Wk, np.float32)
    Wr = np.asarray(Wr, np.float32)
    Wv = np.asarray(Wv, np.float32)
    mix_k = np.asarray(mix_k, np.float32)
    mix_r = np.asarray(mix_r, np.float32)
    lxk = np.asarray(last_x_mix_k, np.float32)
    lxr = np.asarray(last_x_mix_r, np.float32)

    L, dim = x.shape
    tloc = L // NCORES
    key = (dim, tloc, HALO)
    if key not in _cache:
        _cache[key] = _build(dim, tloc, HALO)
    nc = _cache[key]

    in_maps = _prep(x, Wk, Wr, Wv, mix_k, mix_r, lxk, lxr, NCORES, HALO)
    # First execution on a cold device occasionally returns
    # NRT_EXEC_UNIT_UNRECOVERABLE; a retry has always succeeded.
    res = None
    for attempt in range(3):
        try:
            res = run_bass_kernel_spmd(nc, in_maps, core_ids=list(range(NCORES)))
            break
        except Exception:
            if attempt == 2:
                raise

    out = np.empty((L, dim), np.float32)
    for c in range(NCORES):
        out[c * tloc:(c + 1) * tloc, :] = res.results[c]["out"].T
    return out



# revision 3
# speedup vs baseline: 1.1990x; 1.1990x over previous
"""Trainium2 Bass kernel for nn_ChannelMixing (RWKV-style channel mixing).

Math: the reference's FFT decay-conv is the first-order IIR
    h[t] = mix*h[t-1] + x[t],  h[-1] = last_x/(1-mix)
and x_mix = (1-mix)*h, so with weights pre-scaled by (1-mix):
    k = h_k @ (Wk*(1-mix_k)).T,  r = h_r @ (Wr*(1-mix_r)).T
    out = sigmoid(r) * (relu(k)^2 @ Wv.T)

Sharding: time dimension L=4096 split over 8 cores (512 rows each) with a
32-step halo to warm up the scan state (decay <= sigmoid(1) ~ 0.731, so
carry across 32 steps < 5e-5 — far below the bf16 noise floor). Core 0
gets the exact initial state via a per-core init column; no collectives.

Precision: scans keep fp32 state on the DVE and write bf16 (k path) or
fp8-e4m3 (r path) outputs directly. Wk/Wv matmuls run bf16; the Wr
matmul runs fp8 DoubleRow (2x PE throughput, contraction 256/instr);
its 2.5%-rms error is damped ~4x by the sigmoid gate. Measured end-to-end
rel err ~7e-3 vs the 2e-2 gate.

Schedule: Wk runs as two kt-major half-passes over 8 PSUM banks so the
PE paces with the scan cadence (1.26us/tile) instead of starving;
Wr (fp8) and Wv are group-major. Weight DMAs for Wk/Wr ride the sync
queue, Wv weights ride the vector queue, x/outputs the scalar queue.
"""
import numpy as np
import ml_dtypes
from contextlib import ExitStack

import concourse.bass as bass
from concourse import bacc
import concourse.tile as tile
import concourse.mybir as mybir
from concourse.bass_utils import run_bass_kernel_spmd

LEN, DIM = 4096, 2048
NCORES = 8
P = 128
HALO = 32
NT = DIM // P          # 16 channel tiles
TLOC = LEN // NCORES   # 512
TS = TLOC + HALO       # 544

f32 = mybir.dt.float32
bf16 = mybir.dt.bfloat16
fp8 = mybir.dt.float8e4
Alu = mybir.AluOpType
Act = mybir.ActivationFunctionType
DR = mybir.MatmulPerfMode.DoubleRow

_cache = {}


def _build():
    nc = bacc.Bacc(trn_type="TRN2", debug=False)

    xs_d = nc.dram_tensor("xs", [DIM, TS], bf16, kind="ExternalInput").ap()
    # dec image: [P, 2*NT+2]; col 2*ct = mix_k tile ct, 2*ct+1 = mix_r,
    # col 2*NT = 1/s_w (fp8 descale for the r path).
    dec_d = nc.dram_tensor("dec", [P, 2 * NT + 2], f32, kind="ExternalInput").ap()
    ini_d = nc.dram_tensor("ini", [P, 2 * NT], f32, kind="ExternalInput").ap()
    wk_d = nc.dram_tensor("wk", [DIM, DIM], bf16, kind="ExternalInput").ap()
    wv_d = nc.dram_tensor("wv", [DIM, DIM], bf16, kind="ExternalInput").ap()
    wr_d = nc.dram_tensor("wr8", [NT // 2, P, 2, DIM], fp8, kind="ExternalInput").ap()
    out_d = nc.dram_tensor("out", [DIM, TLOC], bf16, kind="ExternalOutput").ap()

    with tile.TileContext(nc) as tc, ExitStack() as ctx:
        const = ctx.enter_context(tc.tile_pool(name="const", bufs=1))
        xs_pool = ctx.enter_context(tc.tile_pool(name="xs", bufs=1))
        hk_pool = ctx.enter_context(tc.tile_pool(name="hk", bufs=1))
        hr_pool = ctx.enter_context(tc.tile_pool(name="hr", bufs=1))
        wk_pool = ctx.enter_context(tc.tile_pool(name="wkp", bufs=6))
        wv_pool = ctx.enter_context(tc.tile_pool(name="wvp", bufs=6))
        w8_pool = ctx.enter_context(tc.tile_pool(name="w8p", bufs=4))
        sq_pool = ctx.enter_context(tc.tile_pool(name="sq", bufs=1))
        sig_pool = ctx.enter_context(tc.tile_pool(name="sg", bufs=1))
        rr_pool = ctx.enter_context(tc.tile_pool(name="rr", bufs=1))
        o_pool = ctx.enter_context(tc.tile_pool(name="o", bufs=3))
        ps_pool = ctx.enter_context(tc.tile_pool(name="ps", bufs=1, space="PSUM"))

        # ---- PE warmup from a memset tile: no DMA dependency, covers the
        # ~2.5us until the first scan output + weight tile land.
        wm = const.tile([P, 512], bf16, name="warm")
        nc.gpsimd.memset(wm[:], 0.25)
        ps_w = ps_pool.tile([P, 512], f32, tag="p7", name="ps_warm")
        for _ in range(16):
            nc.tensor.matmul(ps_w[:], wm[:, 0:P], wm[:], start=True, stop=True)

        dec_t = const.tile([P, 2 * NT + 2], f32)
        nc.scalar.dma_start(dec_t[:], dec_d)
        ini_t = const.tile([P, 2 * NT], f32)
        nc.scalar.dma_start(ini_t[:], ini_d)

        # ---- x slabs (loaded once, shared by both scans) ----
        xs = []
        for ct in range(NT):
            t = xs_pool.tile([P, TS], bf16, tag=f"xs{ct}", name=f"xs{ct}")
            nc.scalar.dma_start(t[:], xs_d[ct * P:(ct + 1) * P, :])
            xs.append(t)

        # ---- scans: fp32 state on DVE; k -> bf16, r -> fp8 DR-packed ----
        hk = []
        for ct in range(NT):
            t = hk_pool.tile([P, TS], bf16, tag=f"hk{ct}", name=f"hk{ct}")
            nc.vector.tensor_tensor_scan(
                t[:], dec_t[:, 2 * ct:2 * ct + 1].broadcast_to([P, TS]),
                xs[ct][:], ini_t[:, 2 * ct:2 * ct + 1],
                op0=Alu.mult, op1=Alu.add)
            hk.append(t)
        hr8 = [hr_pool.tile([P, 2, TS], fp8, tag=f"hr{i}", name=f"hr{i}")
               for i in range(NT // 2)]
        for ct in range(NT):
            c = 2 * ct + 1
            nc.vector.tensor_tensor_scan(
                hr8[ct // 2][:, ct % 2, :],
                dec_t[:, c:c + 1].broadcast_to([P, TS]),
                xs[ct][:], ini_t[:, c:c + 1],
                op0=Alu.mult, op1=Alu.add)

        sq = [sq_pool.tile([P, TLOC], bf16, tag=f"sq{i}", name=f"sq{i}")
              for i in range(NT)]
        sig = [sig_pool.tile([P, TLOC], bf16, tag=f"sg{i}", name=f"sg{i}")
               for i in range(NT)]

        # ---- Wk: two kt-major half-passes, 8 live PSUM groups each ----
        for hp in range(2):
            ps = [ps_pool.tile([P, 512], f32, tag=f"p{m}", name=f"psk{hp}_{m}")
                  for m in range(8)]
            for kt in range(NT):
                wt = wk_pool.tile([P, 1024], bf16, tag="wk", name=f"wk{hp}_{kt}")
                nc.sync.dma_start(
                    wt[:], wk_d[kt * P:(kt + 1) * P, hp * 1024:(hp + 1) * 1024])
                for m in range(8):
                    nc.tensor.matmul(ps[m][:], wt[:, m * P:(m + 1) * P],
                                     hk[kt][:, HALO:],
                                     start=(kt == 0), stop=(kt == NT - 1))
            for m in range(8):
                mi = hp * 8 + m
                rr = rr_pool.tile([P, 512], bf16, tag=f"rr{mi}", name=f"rr{mi}")
                nc.scalar.activation(rr[:], ps[m][:], Act.Relu)
                nc.vector.tensor_mul(sq[mi][:], rr[:], rr[:])

        # ---- Wr: fp8 DoubleRow, group-major ----
        for g in range(4):
            pb = 4 * (g % 2)
            ps = [ps_pool.tile([P, 512], f32, tag=f"p{pb + m}", name=f"psr{g}_{m}")
                  for m in range(4)]
            for kt2 in range(NT // 2):
                wt8 = w8_pool.tile([P, 2, 512], fp8, tag="w8", name=f"wr{g}_{kt2}")
                nc.sync.dma_start(wt8[:], wr_d[kt2, :, :, g * 512:(g + 1) * 512])
                for m in range(4):
                    nc.tensor.matmul(ps[m][:], wt8[:, :, m * P:(m + 1) * P],
                                     hr8[kt2][:, :, HALO:],
                                     start=(kt2 == 0), stop=(kt2 == NT // 2 - 1),
                                     perf_mode=DR)
            for m in range(4):
                nc.scalar.activation(sig[g * 4 + m][:], ps[m][:], Act.Sigmoid,
                                     scale=dec_t[:, 2 * NT:2 * NT + 1])

        # ---- Wv: group-major; gate with sigmoid(r) and stream out ----
        for g in range(4):
            pb = 4 * (g % 2)
            ps = [ps_pool.tile([P, 512], f32, tag=f"p{pb + m}", name=f"psv{g}_{m}")
                  for m in range(4)]
            for kt in range(NT):
                wt = wv_pool.tile([P, 512], bf16, tag="wv", name=f"wv{g}_{kt}")
                nc.gpsimd.dma_start(
                    wt[:], wv_d[kt * P:(kt + 1) * P, g * 512:(g + 1) * 512])
                for m in range(4):
                    nc.tensor.matmul(ps[m][:], wt[:, m * P:(m + 1) * P],
                                     sq[kt][:],
                                     start=(kt == 0), stop=(kt == NT - 1))
            for m in range(4):
                mi = g * 4 + m
                ot = o_pool.tile([P, 512], bf16, tag="ot", name=f"ot{mi}")
                nc.vector.tensor_mul(ot[:], ps[m][:], sig[mi][:])
                nc.scalar.dma_start(out_d[mi * P:(mi + 1) * P, :], ot[:])

    nc.compile()
    return nc


def _sigmoid(v):
    return 1.0 / (1.0 + np.exp(-v.astype(np.float64)))


def _prep(x, Wk, Wr, Wv, mix_k, mix_r, lxk, lxr):
    """Host-side prep: transposes, weight pre-scaling/quant, per-core slabs."""
    mk = _sigmoid(mix_k)
    mr = _sigmoid(mix_r)
    h0k = lxk.astype(np.float64) / (1.0 - mk)
    h0r = lxr.astype(np.float64) / (1.0 - mr)

    dec = np.zeros((P, 2 * NT + 2), np.float32)
    dec[:, 0:2 * NT:2] = mk.astype(np.float32).reshape(NT, P).T
    dec[:, 1:2 * NT:2] = mr.astype(np.float32).reshape(NT, P).T

    wk = np.ascontiguousarray(
        (Wk.T * (1.0 - mk)[:, None]).astype(ml_dtypes.bfloat16))
    wv = np.ascontiguousarray(Wv.T.astype(ml_dtypes.bfloat16))
    wrp = (Wr.T * (1.0 - mr)[:, None]).astype(np.float32)   # [i, o]
    s_w = float(240.0 / np.abs(wrp).max())
    dec[:, 2 * NT] = np.float32(1.0 / s_w)
    wr8 = np.ascontiguousarray(
        (wrp * s_w).reshape(NT // 2, 2, P, DIM).transpose(0, 2, 1, 3)
        .astype(ml_dtypes.float8_e4m3fn))

    xT = x.T.astype(np.float32)                             # [DIM, LEN]
    in_maps = []
    for c in range(NCORES):
        t0 = c * TLOC
        slab = np.empty((DIM, TS), np.float32)
        if c == 0:
            slab[:, :HALO] = 0.0
            bk = h0k * (1.0 / mk) ** HALO
            br = h0r * (1.0 / mr) ** HALO
            ini = np.empty((P, 2 * NT), np.float32)
            ini[:, 0::2] = bk.astype(np.float32).reshape(NT, P).T
            ini[:, 1::2] = br.astype(np.float32).reshape(NT, P).T
        else:
            slab[:, :HALO] = xT[:, t0 - HALO:t0]
            ini = np.zeros((P, 2 * NT), np.float32)
        slab[:, HALO:] = xT[:, t0:t0 + TLOC]
        in_maps.append({
            "xs": slab.astype(ml_dtypes.bfloat16), "dec": dec,
            "ini": np.ascontiguousarray(ini),
            "wk": wk, "wv": wv, "wr8": wr8,
        })
    return in_maps


def kernel(x, Wk, Wr, Wv, mix_k, mix_r, last_x_mix_k, last_x_mix_r):
    x = np.asarray(x, np.float32)
    Wk = np.asarray(Wk, np.float32)
    Wr = np.asarray(Wr, np.float32)
    Wv = np.asarray(Wv, np.float32)

    if "nc" not in _cache:
        _cache["nc"] = _build()
    nc = _cache["nc"]

    in_maps = _prep(x, Wk, Wr, Wv,
                    np.asarray(mix_k, np.float32), np.asarray(mix_r, np.float32),
                    np.asarray(last_x_mix_k, np.float32),
                    np.asarray(last_x_mix_r, np.float32))
    # First execution on a cold device occasionally returns
    # NRT_EXEC_UNIT_UNRECOVERABLE; a retry has always succeeded.
    res = None
    for attempt in range(3):
        try:
            res = run_bass_kernel_spmd(nc, in_maps, core_ids=list(range(NCORES)))
            break
        except Exception:
            if attempt == 2:
                raise

    out = np.empty((LEN, DIM), np.float32)
    for c in range(NCORES):
        out[c * TLOC:(c + 1) * TLOC, :] = res.results[c]["out"].astype(np.float32).T
    return out


# revision 4
# speedup vs baseline: 1.3843x; 1.1545x over previous
"""Trainium2 Bass kernel for nn_ChannelMixing (RWKV-style channel mixing).

Math: the reference's FFT decay-conv is the first-order IIR
    h[t] = mix*h[t-1] + x[t],  h[-1] = last_x/(1-mix)
and x_mix = (1-mix)*h, so with weights pre-scaled by (1-mix):
    k = h_k @ (Wk*(1-mix_k)).T,  r = h_r @ (Wr*(1-mix_r)).T
    out = sigmoid(r) * (relu(k)^2 @ Wv.T)

Sharding: time L=4096 split over 8 cores (512 each) with a 32-step halo
to warm the scan state (decay <= sigmoid(1) ~ 0.731; carry error < 5e-5,
far below the bf16 noise floor). Core 0 gets the exact initial state via
a per-core init column; no collectives.

Precision: scans keep fp32 state on the DVE and write bf16 (k path) or
fp8-e4m3 (r path) outputs directly. Wk/Wv matmuls run bf16; Wr runs fp8
DoubleRow (2x PE throughput, contraction 256/instr); its fp8 error is
damped ~4x by the sigmoid gate. End-to-end rel err ~8e-3 vs 2e-2 gate.

Schedule: Wk runs as two kt-major half-passes over all 8 PSUM banks so
the PE paces with the scan cadence instead of starving; Wr/Wv are
group-major. x slabs ride the sync queue first, then wr8/wv weights
(batched 256-512KB, early-resident); wk weights + outputs ride scalar.
PE cadence is 259ns/512-col matmul on trn2 (2.0 GHz) — the matmul
stream itself is at the hardware floor; the schedule hides the rest.
"""
import numpy as np
import ml_dtypes
from contextlib import ExitStack

import concourse.bass as bass
from concourse import bacc
import concourse.tile as tile
import concourse.mybir as mybir
from concourse.bass_utils import run_bass_kernel_spmd

LEN, DIM = 4096, 2048
NCORES = 8
P = 128
HALO = 32
NT = DIM // P          # 16 channel tiles
TLOC = LEN // NCORES   # 512
TS = TLOC + HALO       # 544

f32 = mybir.dt.float32
bf16 = mybir.dt.bfloat16
fp8 = mybir.dt.float8e4
Alu = mybir.AluOpType
Act = mybir.ActivationFunctionType
DR = mybir.MatmulPerfMode.DoubleRow

_cache = {}


def _build():
    nc = bacc.Bacc(trn_type="TRN2", debug=False)

    xs_d = nc.dram_tensor("xs", [DIM, TS], bf16, kind="ExternalInput").ap()
    # dec image: col 2*ct = mix_k tile ct, 2*ct+1 = mix_r, col 2*NT = 1/s_w.
    dec_d = nc.dram_tensor("dec", [P, 2 * NT + 2], f32, kind="ExternalInput").ap()
    ini_d = nc.dram_tensor("ini", [P, 2 * NT], f32, kind="ExternalInput").ap()
    wk_d = nc.dram_tensor("wk", [DIM, DIM], bf16, kind="ExternalInput").ap()
    # wv pre-permuted host-side to [p, kt, o] so a [128, 4, 512] tile is one DMA.
    wv_d = nc.dram_tensor("wv", [P, NT, DIM], bf16, kind="ExternalInput").ap()
    # wr8 packed [kt4, p, j(4 c-subtiles), o] for fp8 DoubleRow.
    wr_d = nc.dram_tensor("wr8", [NT // 4, P, 4, DIM], fp8, kind="ExternalInput").ap()
    # out as [p, m, t]; host reassembles.
    out_d = nc.dram_tensor("out", [P, NT, TLOC], bf16, kind="ExternalOutput").ap()

    with tile.TileContext(nc) as tc, ExitStack() as ctx:
        const = ctx.enter_context(tc.tile_pool(name="const", bufs=1))
        xs_pool = ctx.enter_context(tc.tile_pool(name="xs", bufs=1))
        hk_pool = ctx.enter_context(tc.tile_pool(name="hk", bufs=1))
        hr_pool = ctx.enter_context(tc.tile_pool(name="hr", bufs=1))
        wk_pool = ctx.enter_context(tc.tile_pool(name="wkp", bufs=6))
        wv_pool = ctx.enter_context(tc.tile_pool(name="wvp", bufs=8))
        w8_pool = ctx.enter_context(tc.tile_pool(name="w8p", bufs=1))
        sq_pool = ctx.enter_context(tc.tile_pool(name="sq", bufs=1))
        sig_pool = ctx.enter_context(tc.tile_pool(name="sg", bufs=1))
        rr_pool = ctx.enter_context(tc.tile_pool(name="rr", bufs=1))
        o_pool = ctx.enter_context(tc.tile_pool(name="o", bufs=3))
        ps_pool = ctx.enter_context(tc.tile_pool(name="ps", bufs=1, space="PSUM"))

        # ---- PE warmup from a memset tile: no DMA dependency.
        wm = const.tile([P, 512], bf16, name="warm")
        nc.vector.memset(wm[:], 0.25)
        ps_w = ps_pool.tile([P, 512], f32, tag="p7", name="ps_warm")
        for _ in range(14):
            nc.tensor.matmul(ps_w[:], wm[:, 0:P], wm[:], start=True, stop=True)

        dec_t = const.tile([P, 2 * NT + 2], f32)
        nc.scalar.dma_start(dec_t[:], dec_d)
        ini_t = const.tile([P, 2 * NT], f32)
        nc.scalar.dma_start(ini_t[:], ini_d)

        # ---- x slabs on the sync queue (first), shared by both scans ----
        xs = []
        for ct in range(NT):
            t = xs_pool.tile([P, TS], bf16, tag=f"xs{ct}", name=f"xs{ct}")
            nc.sync.dma_start(t[:], xs_d[ct * P:(ct + 1) * P, :])
            xs.append(t)

        # wr8 weights: 16 tiles, early-resident (4MB), on sync after xs.
        w8 = []
        for g in range(4):
            for kt4 in range(NT // 4):
                t = w8_pool.tile([P, 4, 512], fp8, tag=f"w8_{g}_{kt4}",
                                 name=f"wr{g}_{kt4}")
                nc.sync.dma_start(t[:], wr_d[kt4, :, :, g * 512:(g + 1) * 512])
                w8.append(t)

        # ---- scans: fp32 state on DVE; k -> bf16, r -> fp8 DR-packed ----
        hk = []
        for ct in range(NT):
            t = hk_pool.tile([P, TS], bf16, tag=f"hk{ct}", name=f"hk{ct}")
            nc.vector.tensor_tensor_scan(
                t[:], dec_t[:, 2 * ct:2 * ct + 1].broadcast_to([P, TS]),
                xs[ct][:], ini_t[:, 2 * ct:2 * ct + 1],
                op0=Alu.mult, op1=Alu.add)
            hk.append(t)
        hr8 = [hr_pool.tile([P, 2, TS], fp8, tag=f"hr{i}", name=f"hr{i}")
               for i in range(NT // 2)]
        for ct in range(NT):
            c = 2 * ct + 1
            nc.vector.tensor_tensor_scan(
                hr8[ct // 2][:, ct % 2, :],
                dec_t[:, c:c + 1].broadcast_to([P, TS]),
                xs[ct][:], ini_t[:, c:c + 1],
                op0=Alu.mult, op1=Alu.add)

        sq = [sq_pool.tile([P, TLOC], bf16, tag=f"sq{i}", name=f"sq{i}")
              for i in range(NT)]
        sig = [sig_pool.tile([P, TLOC], bf16, tag=f"sg{i}", name=f"sg{i}")
               for i in range(NT)]

        # ---- Wk: two kt-major half-passes, 8 live PSUM groups each ----
        for hp in range(2):
            ps = [ps_pool.tile([P, 512], f32, tag=f"p{m}", name=f"psk{hp}_{m}")
                  for m in range(8)]
            for kt in range(NT):
                wt = wk_pool.tile([P, 1024], bf16, tag="wk", name=f"wk{hp}_{kt}")
                nc.scalar.dma_start(
                    wt[:], wk_d[kt * P:(kt + 1) * P, hp * 1024:(hp + 1) * 1024])
                for m in range(8):
                    nc.tensor.matmul(ps[m][:], wt[:, m * P:(m + 1) * P],
                                     hk[kt][:, HALO:],
                                     start=(kt == 0), stop=(kt == NT - 1))
            for m in range(8):
                mi = hp * 8 + m
                rr = rr_pool.tile([P, 512], bf16, tag=f"rr{mi}", name=f"rr{mi}")
                nc.scalar.activation(rr[:], ps[m][:], Act.Relu)
                nc.vector.tensor_mul(sq[mi][:], rr[:], rr[:])

        # ---- Wr: fp8 DoubleRow, group-major ----
        for g in range(4):
            pb = 4 * (g % 2)
            ps = [ps_pool.tile([P, 512], f32, tag=f"p{pb + m}", name=f"psr{g}_{m}")
                  for m in range(4)]
            for kt4 in range(NT // 4):
                wt8 = w8[g * 4 + kt4]
                for half in range(2):
                    kt2 = 2 * kt4 + half
                    for m in range(4):
                        nc.tensor.matmul(
                            ps[m][:], wt8[:, 2 * half:2 * half + 2, m * P:(m + 1) * P],
                            hr8[kt2][:, :, HALO:],
                            start=(kt2 == 0), stop=(kt2 == NT // 2 - 1),
                            perf_mode=DR)
            for m in range(4):
                nc.scalar.activation(sig[g * 4 + m][:], ps[m][:], Act.Sigmoid,
                                     scale=dec_t[:, 2 * NT:2 * NT + 1])

        # ---- Wv: group-major; gate with sigmoid(r) and stream out ----
        for g in range(4):
            pb = 4 * (g % 2)
            ps = [ps_pool.tile([P, 512], f32, tag=f"p{pb + m}", name=f"psv{g}_{m}")
                  for m in range(4)]
            for kt4 in range(NT // 4):
                wt = wv_pool.tile([P, 4, 512], bf16, tag="wv", name=f"wv{g}_{kt4}")
                nc.sync.dma_start(wt[:], wv_d[:, 4 * kt4:4 * kt4 + 4,
                                              g * 512:(g + 1) * 512])
                for j in range(4):
                    kt = 4 * kt4 + j
                    for m in range(4):
                        nc.tensor.matmul(ps[m][:], wt[:, j, m * P:(m + 1) * P],
                                         sq[kt][:],
                                         start=(kt == 0), stop=(kt == NT - 1))
            for mp in range(2):
                ot = o_pool.tile([P, 2, 512], bf16, tag="ot", name=f"ot{g}_{mp}")
                for m in (2 * mp, 2 * mp + 1):
                    mi = g * 4 + m
                    nc.vector.tensor_mul(ot[:, m - 2 * mp, :], ps[m][:], sig[mi][:])
                nc.scalar.dma_start(
                    out_d[:, g * 4 + 2 * mp:g * 4 + 2 * mp + 2, :], ot[:])

    nc.compile()
    return nc


def _sigmoid(v):
    return 1.0 / (1.0 + np.exp(-v.astype(np.float64)))


def _prep(x, Wk, Wr, Wv, mix_k, mix_r, lxk, lxr):
    """Host-side prep: transposes, weight pre-scaling/quant, per-core slabs."""
    mk = _sigmoid(mix_k)
    mr = _sigmoid(mix_r)
    h0k = lxk.astype(np.float64) / (1.0 - mk)
    h0r = lxr.astype(np.float64) / (1.0 - mr)

    dec = np.zeros((P, 2 * NT + 2), np.float32)
    dec[:, 0:2 * NT:2] = mk.astype(np.float32).reshape(NT, P).T
    dec[:, 1:2 * NT:2] = mr.astype(np.float32).reshape(NT, P).T

    wk = np.ascontiguousarray(
        (Wk.T * (1.0 - mk)[:, None]).astype(ml_dtypes.bfloat16))
    # wv: [i, o] -> [p, kt, o]
    wv = np.ascontiguousarray(
        Wv.T.astype(ml_dtypes.bfloat16).reshape(NT, P, DIM).transpose(1, 0, 2))
    wrp = (Wr.T * (1.0 - mr)[:, None]).astype(np.float32)   # [i, o]
    s_w = float(240.0 / np.abs(wrp).max())
    dec[:, 2 * NT] = np.float32(1.0 / s_w)
    wr8 = np.ascontiguousarray(
        (wrp * s_w).reshape(NT // 4, 4, P, DIM).transpose(0, 2, 1, 3)
        .astype(ml_dtypes.float8_e4m3fn))

    xT = x.T.astype(np.float32)                             # [DIM, LEN]
    in_maps = []
    for c in range(NCORES):
        t0 = c * TLOC
        slab = np.empty((DIM, TS), np.float32)
        if c == 0:
            slab[:, :HALO] = 0.0
            bk = h0k * (1.0 / mk) ** HALO
            br = h0r * (1.0 / mr) ** HALO
            ini = np.empty((P, 2 * NT), np.float32)
            ini[:, 0::2] = bk.astype(np.float32).reshape(NT, P).T
            ini[:, 1::2] = br.astype(np.float32).reshape(NT, P).T
        else:
            slab[:, :HALO] = xT[:, t0 - HALO:t0]
            ini = np.zeros((P, 2 * NT), np.float32)
        slab[:, HALO:] = xT[:, t0:t0 + TLOC]
        in_maps.append({
            "xs": slab.astype(ml_dtypes.bfloat16), "dec": dec,
            "ini": np.ascontiguousarray(ini),
            "wk": wk, "wv": wv, "wr8": wr8,
        })
    return in_maps


def kernel(x, Wk, Wr, Wv, mix_k, mix_r, last_x_mix_k, last_x_mix_r):
    x = np.asarray(x, np.float32)
    Wk = np.asarray(Wk, np.float32)
    Wr = np.asarray(Wr, np.float32)
    Wv = np.asarray(Wv, np.float32)

    if "nc" not in _cache:
        _cache["nc"] = _build()
    nc = _cache["nc"]

    in_maps = _prep(x, Wk, Wr, Wv,
                    np.asarray(mix_k, np.float32), np.asarray(mix_r, np.float32),
                    np.asarray(last_x_mix_k, np.float32),
                    np.asarray(last_x_mix_r, np.float32))
    # First execution on a cold device occasionally returns
    # NRT_EXEC_UNIT_UNRECOVERABLE; a retry has always succeeded.
    res = None
    for attempt in range(3):
        try:
            res = run_bass_kernel_spmd(nc, in_maps, core_ids=list(range(NCORES)))
            break
        except Exception:
            if attempt == 2:
                raise

    out = np.empty((LEN, DIM), np.float32)
    for c in range(NCORES):
        o = res.results[c]["out"].astype(np.float32)        # [p, m, t]
        out[c * TLOC:(c + 1) * TLOC, :] = o.transpose(1, 0, 2).reshape(DIM, TLOC).T
    return out


# revision 7
# speedup vs baseline: 1.4635x; 1.0572x over previous
"""Trainium2 Bass kernel for nn_ChannelMixing (RWKV-style channel mixing).

Math: the reference's FFT decay-conv is the first-order IIR
    h[t] = mix*h[t-1] + x[t],  h[-1] = last_x/(1-mix)
and x_mix = (1-mix)*h, so with weights pre-scaled by (1-mix):
    k = h_k @ (Wk*(1-mix_k)).T,  r = h_r @ (Wr*(1-mix_r)).T
    out = sigmoid(r) * (relu(k)^2 @ Wv.T)

Sharding: time L=4096 split over 8 cores (512 each) with a 32-step halo
to warm the scan state (decay <= sigmoid(1) ~ 0.731; carry error < 5e-5,
far below the bf16 noise floor). Core 0 gets the exact initial state via
a per-core init column; no collectives.

Precision: scans keep fp32 state on the DVE and write bf16 (k path) or
fp8-e4m3 (r path) outputs directly. Wk/Wv matmuls run bf16; Wr runs fp8
DoubleRow (2x PE throughput, contraction 256/instr); its fp8 error is
damped ~4x by the sigmoid gate. End-to-end rel err ~8e-3 vs 2e-2 gate.

Schedule: Wk runs as two kt-major half-passes over all 8 PSUM banks so
the PE paces with the scan cadence instead of starving; Wr/Wv are
group-major. x slabs ride the sync queue first, then wr8/wv weights
(batched 256-512KB, early-resident); wk weights + outputs ride scalar.
PE cadence is 259ns/512-col matmul on trn2 (2.0 GHz) — the matmul
stream itself is at the hardware floor; the schedule hides the rest.
"""
import numpy as np
import ml_dtypes
from contextlib import ExitStack

import concourse.bass as bass
from concourse import bacc
import concourse.tile as tile
import concourse.mybir as mybir
from concourse.bass_utils import run_bass_kernel_spmd

LEN, DIM = 4096, 2048
NCORES = 8
P = 128
HALO = 32
NT = DIM // P          # 16 channel tiles
TLOC = LEN // NCORES   # 512
TS = TLOC + HALO       # 544

f32 = mybir.dt.float32
bf16 = mybir.dt.bfloat16
fp8 = mybir.dt.float8e4
Alu = mybir.AluOpType
Act = mybir.ActivationFunctionType
DR = mybir.MatmulPerfMode.DoubleRow

_cache = {}


def _build():
    nc = bacc.Bacc(trn_type="TRN2", debug=False)

    # x pre-packed host-side into a [p, ct*TS] SBUF image: 17KB contiguous
    # rows -> large DMA packets (the [DIM, TS] layout shattered into 136B
    # packets and starved the scans).
    xs_d = nc.dram_tensor("xs", [P, NT * TS], bf16, kind="ExternalInput").ap()
    # dec image: col 2*ct = mix_k tile ct, 2*ct+1 = mix_r, col 2*NT = 1/s_w.
    dec_d = nc.dram_tensor("dec", [P, 2 * NT + 2], f32, kind="ExternalInput").ap()
    ini_d = nc.dram_tensor("ini", [P, 2 * NT], f32, kind="ExternalInput").ap()
    wk_d = nc.dram_tensor("wk", [DIM, DIM], bf16, kind="ExternalInput").ap()
    # wv pre-permuted host-side to [p, kt, o] so a [128, 4, 512] tile is one DMA.
    wv_d = nc.dram_tensor("wv", [P, NT, DIM], bf16, kind="ExternalInput").ap()
    # wr8 packed [kt4, p, j(4 c-subtiles), o] for fp8 DoubleRow.
    wr_d = nc.dram_tensor("wr8", [NT // 4, P, 4, DIM], fp8, kind="ExternalInput").ap()
    # out as [p, m, t]; host reassembles.
    out_d = nc.dram_tensor("out", [P, NT, TLOC], bf16, kind="ExternalOutput").ap()

    with tile.TileContext(nc) as tc, ExitStack() as ctx:
        const = ctx.enter_context(tc.tile_pool(name="const", bufs=1))
        xs_pool = ctx.enter_context(tc.tile_pool(name="xs", bufs=1))
        hk_pool = ctx.enter_context(tc.tile_pool(name="hk", bufs=1))
        hr_pool = ctx.enter_context(tc.tile_pool(name="hr", bufs=1))
        wk_pool = ctx.enter_context(tc.tile_pool(name="wkp", bufs=6))
        wv_pool = ctx.enter_context(tc.tile_pool(name="wvp", bufs=8))
        w8_pool = ctx.enter_context(tc.tile_pool(name="w8p", bufs=1))
        sq_pool = ctx.enter_context(tc.tile_pool(name="sq", bufs=1))
        sig_pool = ctx.enter_context(tc.tile_pool(name="sg", bufs=1))
        rr_pool = ctx.enter_context(tc.tile_pool(name="rr", bufs=1))
        o_pool = ctx.enter_context(tc.tile_pool(name="o", bufs=3))
        ps_pool = ctx.enter_context(tc.tile_pool(name="ps", bufs=1, space="PSUM"))

        # ---- PE warmup from a memset tile: no DMA dependency.
        wm = const.tile([P, 512], bf16, name="warm")
        nc.vector.memset(wm[:], 0.25)
        ps_w = ps_pool.tile([P, 512], f32, tag="p7", name="ps_warm")
        for _ in range(12):
            nc.tensor.matmul(ps_w[:], wm[:, 0:P], wm[:], start=True, stop=True)

        # dec/ini on the gpsimd queue: it has no ACT-table preamble, so they
        # land earliest and unblock scan 0.
        dec_t = const.tile([P, 2 * NT + 2], f32)
        nc.gpsimd.dma_start(dec_t[:], dec_d)
        ini_t = const.tile([P, 2 * NT], f32)
        nc.gpsimd.dma_start(ini_t[:], ini_d)

        # ---- x slabs on the sync queue (first), shared by both scans ----
        xs2 = []
        for i in range(NT // 2):
            t = xs_pool.tile([P, 2 * TS], bf16, tag=f"xs{i}", name=f"xs{i}")
            nc.sync.dma_start(t[:], xs_d[:, 2 * i * TS:(2 * i + 2) * TS])
            xs2.append(t)
        xs = [xs2[ct // 2][:, (ct % 2) * TS:(ct % 2 + 1) * TS] for ct in range(NT)]

        # wr8 weights: 16 tiles, early-resident (4MB), on sync after xs.
        w8 = []
        for g in range(4):
            for kt4 in range(NT // 4):
                t = w8_pool.tile([P, 4, 512], fp8, tag=f"w8_{g}_{kt4}",
                                 name=f"wr{g}_{kt4}")
                nc.sync.dma_start(t[:], wr_d[kt4, :, :, g * 512:(g + 1) * 512])
                w8.append(t)

        # ---- scans: fp32 state on DVE; k -> bf16, r -> fp8 DR-packed ----
        hk = []
        for ct in range(NT):
            t = hk_pool.tile([P, TS], bf16, tag=f"hk{ct}", name=f"hk{ct}")
            nc.vector.tensor_tensor_scan(
                t[:], dec_t[:, 2 * ct:2 * ct + 1].broadcast_to([P, TS]),
                xs[ct], ini_t[:, 2 * ct:2 * ct + 1],
                op0=Alu.mult, op1=Alu.add)
            hk.append(t)
        hr8 = [hr_pool.tile([P, 2, TS], fp8, tag=f"hr{i}", name=f"hr{i}")
               for i in range(NT // 2)]
        for ct in range(NT):
            c = 2 * ct + 1
            nc.vector.tensor_tensor_scan(
                hr8[ct // 2][:, ct % 2, :],
                dec_t[:, c:c + 1].broadcast_to([P, TS]),
                xs[ct], ini_t[:, c:c + 1],
                op0=Alu.mult, op1=Alu.add)

        sq = [sq_pool.tile([P, TLOC], bf16, tag=f"sq{i}", name=f"sq{i}")
              for i in range(NT)]
        sig = [sig_pool.tile([P, TLOC], bf16, tag=f"sg{i}", name=f"sg{i}")
               for i in range(NT)]

        # ---- Wk: two kt-major half-passes, 8 live PSUM groups each ----
        for hp in range(2):
            ps = [ps_pool.tile([P, 512], f32, tag=f"p{m}", name=f"psk{hp}_{m}")
                  for m in range(8)]
            for kt in range(NT):
                wt = wk_pool.tile([P, 1024], bf16, tag="wk", name=f"wk{hp}_{kt}")
                nc.scalar.dma_start(
                    wt[:], wk_d[kt * P:(kt + 1) * P, hp * 1024:(hp + 1) * 1024])
                for m in range(8):
                    nc.tensor.matmul(ps[m][:], wt[:, m * P:(m + 1) * P],
                                     hk[kt][:, HALO:],
                                     start=(kt == 0), stop=(kt == NT - 1))
            for m in range(8):
                mi = hp * 8 + m
                rr = rr_pool.tile([P, 512], bf16, tag=f"rr{mi}", name=f"rr{mi}")
                nc.scalar.activation(rr[:], ps[m][:], Act.Relu)
                nc.vector.tensor_mul(sq[mi][:], rr[:], rr[:])

        # ---- Wr: fp8 DoubleRow, group-major ----
        for g in range(4):
            pb = 4 * (g % 2)
            ps = [ps_pool.tile([P, 512], f32, tag=f"p{pb + m}", name=f"psr{g}_{m}")
                  for m in range(4)]
            for kt4 in range(NT // 4):
                wt8 = w8[g * 4 + kt4]
                for half in range(2):
                    kt2 = 2 * kt4 + half
                    for m in range(4):
                        nc.tensor.matmul(
                            ps[m][:], wt8[:, 2 * half:2 * half + 2, m * P:(m + 1) * P],
                            hr8[kt2][:, :, HALO:],
                            start=(kt2 == 0), stop=(kt2 == NT // 2 - 1),
                            perf_mode=DR)
            for m in range(4):
                nc.scalar.activation(sig[g * 4 + m][:], ps[m][:], Act.Sigmoid,
                                     scale=dec_t[:, 2 * NT:2 * NT + 1])

        # ---- Wv: group-major; gate with sigmoid(r) and stream out ----
        for g in range(4):
            pb = 4 * (g % 2)
            ps = [ps_pool.tile([P, 512], f32, tag=f"p{pb + m}", name=f"psv{g}_{m}")
                  for m in range(4)]
            for kt4 in range(NT // 4):
                wt = wv_pool.tile([P, 4, 512], bf16, tag="wv", name=f"wv{g}_{kt4}")
                nc.sync.dma_start(wt[:], wv_d[:, 4 * kt4:4 * kt4 + 4,
                                              g * 512:(g + 1) * 512])
                for j in range(4):
                    kt = 4 * kt4 + j
                    for m in range(4):
                        nc.tensor.matmul(ps[m][:], wt[:, j, m * P:(m + 1) * P],
                                         sq[kt][:],
                                         start=(kt == 0), stop=(kt == NT - 1))
            for mp in range(2):
                ot = o_pool.tile([P, 2, 512], bf16, tag="ot", name=f"ot{g}_{mp}")
                for m in (2 * mp, 2 * mp + 1):
                    mi = g * 4 + m
                    nc.vector.tensor_mul(ot[:, m - 2 * mp, :], ps[m][:], sig[mi][:])
                nc.sync.dma_start(
                    out_d[:, g * 4 + 2 * mp:g * 4 + 2 * mp + 2, :], ot[:])

    nc.compile()
    return nc


def _sigmoid(v):
    return 1.0 / (1.0 + np.exp(-v.astype(np.float64)))


def _prep(x, Wk, Wr, Wv, mix_k, mix_r, lxk, lxr):
    """Host-side prep: transposes, weight pre-scaling/quant, per-core slabs."""
    mk = _sigmoid(mix_k)
    mr = _sigmoid(mix_r)
    h0k = lxk.astype(np.float64) / (1.0 - mk)
    h0r = lxr.astype(np.float64) / (1.0 - mr)

    dec = np.zeros((P, 2 * NT + 2), np.float32)
    dec[:, 0:2 * NT:2] = mk.astype(np.float32).reshape(NT, P).T
    dec[:, 1:2 * NT:2] = mr.astype(np.float32).reshape(NT, P).T

    wk = np.ascontiguousarray(
        (Wk.T * (1.0 - mk)[:, None]).astype(ml_dtypes.bfloat16))
    # wv: [i, o] -> [p, kt, o]
    wv = np.ascontiguousarray(
        Wv.T.astype(ml_dtypes.bfloat16).reshape(NT, P, DIM).transpose(1, 0, 2))
    wrp = (Wr.T * (1.0 - mr)[:, None]).astype(np.float32)   # [i, o]
    s_w = float(240.0 / np.abs(wrp).max())
    dec[:, 2 * NT] = np.float32(1.0 / s_w)
    wr8 = np.ascontiguousarray(
        (wrp * s_w).reshape(NT // 4, 4, P, DIM).transpose(0, 2, 1, 3)
        .astype(ml_dtypes.float8_e4m3fn))

    xT = x.T.astype(np.float32)                             # [DIM, LEN]
    in_maps = []
    for c in range(NCORES):
        t0 = c * TLOC
        slab = np.empty((DIM, TS), np.float32)
        if c == 0:
            slab[:, :HALO] = 0.0
            bk = h0k * (1.0 / mk) ** HALO
            br = h0r * (1.0 / mr) ** HALO
            ini = np.empty((P, 2 * NT), np.float32)
            ini[:, 0::2] = bk.astype(np.float32).reshape(NT, P).T
            ini[:, 1::2] = br.astype(np.float32).reshape(NT, P).T
        else:
            slab[:, :HALO] = xT[:, t0 - HALO:t0]
            ini = np.zeros((P, 2 * NT), np.float32)
        slab[:, HALO:] = xT[:, t0:t0 + TLOC]
        img = slab.reshape(NT, P, TS).transpose(1, 0, 2).reshape(P, NT * TS)
        in_maps.append({
            "xs": np.ascontiguousarray(img.astype(ml_dtypes.bfloat16)), "dec": dec,
            "ini": np.ascontiguousarray(ini),
            "wk": wk, "wv": wv, "wr8": wr8,
        })
    return in_maps


def kernel(x, Wk, Wr, Wv, mix_k, mix_r, last_x_mix_k, last_x_mix_r):
    x = np.asarray(x, np.float32)
    Wk = np.asarray(Wk, np.float32)
    Wr = np.asarray(Wr, np.float32)
    Wv = np.asarray(Wv, np.float32)

    if "nc" not in _cache:
        _cache["nc"] = _build()
    nc = _cache["nc"]

    in_maps = _prep(x, Wk, Wr, Wv,
                    np.asarray(mix_k, np.float32), np.asarray(mix_r, np.float32),
                    np.asarray(last_x_mix_k, np.float32),
                    np.asarray(last_x_mix_r, np.float32))
    # First execution on a cold device occasionally returns
    # NRT_EXEC_UNIT_UNRECOVERABLE; a retry has always succeeded.
    res = None
    for attempt in range(3):
        try:
            res = run_bass_kernel_spmd(nc, in_maps, core_ids=list(range(NCORES)))
            break
        except Exception:
            if attempt == 2:
                raise

    out = np.empty((LEN, DIM), np.float32)
    for c in range(NCORES):
        o = res.results[c]["out"].astype(np.float32)        # [p, m, t]
        out[c * TLOC:(c + 1) * TLOC, :] = o.transpose(1, 0, 2).reshape(DIM, TLOC).T
    return out
